# revision 1
# baseline (speedup 1.0000x reference)
"""Trainium2 Bass kernel for nn_CortexNetwork (dense_cnn, memory-bound).

Reference computation:
    patches[c,i,j,u,v] = x[c, rx[i]+u, ry[j]+v]
    aff[i,j] = sum_{c,u,v} patches * Wa
    exc[i,j] = sum_c prev[c,i,j] * sum_{x,y} We[c,i,j,x,y]   (inh likewise, Wi)
    out      = broadcast_c(relu(aff + 0.9*exc - 0.9*inh))

Strategy: tensor-parallel over the 36x36=1296 grid units, 162 units per
core on 8 cores; every reduction is unit-local so there are no
collectives.  The host lays each core's data out as 20 tiles of
[128 partitions = 16 channels x 8 units,
 3744 columns  = We(1296) | -Wi(1296) | Wa(576) | patch(576)]
plus one 32-partition tile for the 2 leftover units, so the device sees
one linear ~1.9MB DMA per tile.  Wi is negated on the host so the whole
lateral term is one reduction: 0.9*prev * sum(We|-Wi).  The free-dim
reductions are split across ScalarE (activation with scale=0.9*prev and
accum_out) and VectorE (tensor_reduce + per-partition multiply), with
ownership interleaved over tiles so both engines drain with the DMA
stream; all afferent products run on VectorE.  The final sum over the
16 channel partitions is a 0/1-selector matmul on the tensor engine,
then relu.
"""

import numpy as np

import concourse.bass as bass
import concourse.bacc as bacc
import concourse.mybir as mybir
from concourse import tile
from concourse.bass_utils import run_bass_kernel_spmd

N_CORES = 8
C = 16
GX = GY = 36
RF = 24
IMG = 64
GAMMA = 0.9

UNITS = GX * GY                  # 1296
PER_CORE = UNITS // N_CORES      # 162
S = 8                            # units per full tile (partition dim C*S=128)
TF = PER_CORE // S               # 20 full tiles
S2 = PER_CORE - TF * S           # 2 units in the last (32-partition) tile
T = TF + 1                       # 21 tiles total
FW = GX * GY                     # lateral free size per channel: 1296
FA = RF * RF                     # afferent free size per channel: 576
COLS = 2 * FW + 2 * FA           # 3744
# Full tiles whose lateral reduction runs on VectorE, spread through the
# stream so ScalarE and VectorE drain together; the rest go to ScalarE.
DVE_TILES = (2, 6, 9, 13, 16, 18)

_PROGRAM_CACHE = {}


def _build_program():
    f32 = mybir.dt.float32
    AL = mybir.AluOpType
    AF = mybir.ActivationFunctionType
    AX = mybir.AxisListType

    nc = bacc.Bacc(
        "TRN2", target_bir_lowering=False, debug=False, num_devices=N_CORES
    )
    big = nc.dram_tensor("big", [TF, 128, COLS], f32, kind="ExternalInput").ap()
    big2_d = nc.dram_tensor("big2", [C * S2, COLS], f32, kind="ExternalInput").ap()
    possb_d = nc.dram_tensor("possb", [128, TF], f32, kind="ExternalInput").ap()
    possb2_d = nc.dram_tensor("possb2", [C * S2, 1], f32, kind="ExternalInput").ap()
    sel_d = nc.dram_tensor("sel", [128, S], f32, kind="ExternalInput").ap()
    sel2_d = nc.dram_tensor("sel2", [C * S2, S2], f32, kind="ExternalInput").ap()
    out_d = nc.dram_tensor("out", [S, T], f32, kind="ExternalOutput").ap()

    with tile.TileContext(nc) as tc:
        with (
            tc.tile_pool(name="w", bufs=8) as wp,
            tc.tile_pool(name="w2", bufs=1) as wp2,
            tc.tile_pool(name="cst", bufs=1) as cp,
            tc.tile_pool(name="junk", bufs=3) as jp,
            tc.tile_pool(name="acc", bufs=3) as accp,
            tc.tile_pool(name="fin", bufs=1) as fp,
            tc.tile_pool(name="ps", bufs=1, space="PSUM") as pp,
        ):
            possb = cp.tile([128, TF], f32, tag="possb")
            possb2 = cp.tile([C * S2, 1], f32, tag="possb2")
            sel = cp.tile([128, S], f32, tag="sel")
            sel2 = cp.tile([C * S2, S2], f32, tag="sel2")
            # partials: lateral col + afferent col per tile
            plat = cp.tile([128, TF], f32, tag="plat")
            paff = cp.tile([128, TF], f32, tag="paff")
            p2 = cp.tile([C * S2, 2], f32, tag="p2")
            nc.gpsimd.dma_start(possb[:], possb_d[:])
            nc.gpsimd.dma_start(possb2[:], possb2_d[:])
            nc.gpsimd.dma_start(sel[:], sel_d[:])
            nc.gpsimd.dma_start(sel2[:], sel2_d[:])

            def lateral_act(w, scale_ap, out_col):
                # one ScalarE op over the merged We|-Wi region
                j = jp.tile([128, 2 * FW], f32, tag="jlat")
                nc.scalar.activation(
                    j[:w.shape[0], :], w[:, 0:2 * FW], AF.Copy,
                    scale=scale_ap, accum_out=out_col,
                )

            def lateral_dve(w, scale_ap, out_col):
                r = accp.tile([128, 1], f32, tag="r")
                nc.vector.tensor_reduce(
                    r[:w.shape[0], :], w[:, 0:2 * FW], axis=AX.X, op=AL.add
                )
                nc.vector.tensor_mul(out_col, r[:w.shape[0], :], scale_ap)

            def afferent(w, out_col):
                prod = jp.tile([128, FA], f32, tag="prod")
                nc.vector.tensor_mul(
                    prod[:w.shape[0], :], w[:, 2 * FW:2 * FW + FA],
                    w[:, 2 * FW + FA:COLS],
                )
                nc.vector.tensor_reduce(
                    out_col, prod[:w.shape[0], :], axis=AX.X, op=AL.add
                )

            # The 32-partition leftover tile transfers slowly (few DMA
            # engines cover 32 partitions), so put it FIRST on the sync
            # HWDGE FIFO — FIFO order guarantees it streams before the
            # full tiles instead of trickling after them.
            w2 = wp2.tile([C * S2, COLS], f32, tag="w2")
            nc.sync.dma_start(w2[:], big2_d[:])
            lateral_act(w2, possb2[:, 0:1], p2[:, 0:1])
            afferent(w2, p2[:, 1:2])

            for t in range(TF):
                w = wp.tile([128, COLS], f32, tag="w")
                nc.sync.dma_start(w[:], big[t])
                if t in DVE_TILES:
                    lateral_dve(w, possb[:, t:t + 1], plat[:, t:t + 1])
                else:
                    lateral_act(w, possb[:, t:t + 1], plat[:, t:t + 1])
                afferent(w, paff[:, t:t + 1])

            # Channel sum via 0/1-selector matmuls on PE; lateral and
            # afferent partials accumulate into the same PSUM region.
            psum = pp.tile([S, TF], f32, tag="ps")
            psum2 = pp.tile([S2, 1], f32, tag="ps2")
            nc.tensor.matmul(psum[:], sel[:], plat[:], start=True, stop=False)
            nc.tensor.matmul(psum[:], sel[:], paff[:], start=False, stop=True)
            nc.tensor.matmul(psum2[:], sel2[:], p2[:, 0:1],
                             start=True, stop=False)
            nc.tensor.matmul(psum2[:], sel2[:], p2[:, 1:2],
                             start=False, stop=True)

            res = fp.tile([S, T], f32, tag="res")
            nc.vector.memset(res[:], 0.0)
            nc.vector.tensor_scalar_max(res[:, 0:TF], psum[:], 0.0)
            nc.vector.tensor_scalar_max(res[0:S2, TF:T], psum2[:], 0.0)
            nc.sync.dma_start(out_d[:], res[:])

    nc.compile()
    return nc


def _get_program():
    if "nc" not in _PROGRAM_CACHE:
        _PROGRAM_CACHE["nc"] = _build_program()
    return _PROGRAM_CACHE["nc"]


def _prep_in_maps(inputs):
    x = np.asarray(inputs["x"], dtype=np.float32)
    prev = np.asarray(inputs["prev_activity"], dtype=np.float32)
    wa = np.asarray(inputs["afferent_weights"], dtype=np.float32).reshape(C, UNITS, FA)
    we = np.asarray(inputs["ex_lateral_weights"], dtype=np.float32).reshape(C, UNITS, FW)
    wi = np.asarray(inputs["in_lateral_weights"], dtype=np.float32).reshape(C, UNITS, FW)
    rx = np.asarray(inputs["rx"]).astype(np.int64)
    ry = np.asarray(inputs["ry"]).astype(np.int64)

    u = np.arange(RF)
    ix = rx[:, None] + u                     # [GX, RF]
    iy = ry[:, None] + u                     # [GY, RF]
    px = x[:, ix, :]                         # [C, GX, RF, IMG]
    patches = px[:, :, :, iy]                # [C, GX, RF, GY, RF]
    patches = np.ascontiguousarray(patches.transpose(0, 1, 3, 2, 4))
    patches = patches.reshape(C, UNITS, FA)
    prevf = prev.reshape(C, UNITS)

    sel = (np.arange(128)[:, None] % S == np.arange(S)[None, :]).astype(np.float32)
    sel2 = (np.arange(C * S2)[:, None] % S2 == np.arange(S2)[None, :]).astype(np.float32)
    blk = np.concatenate([we, -wi, wa, patches], axis=2)   # [C, UNITS, COLS]

    in_maps = []
    for k in range(N_CORES):
        n0 = k * PER_CORE
        s = blk[:, n0:n0 + TF * S]                          # [C, 160, COLS]
        big = s.reshape(C, TF, S, COLS).transpose(1, 0, 2, 3).reshape(TF, C * S, COLS)
        big2 = blk[:, n0 + TF * S:n0 + PER_CORE].reshape(C * S2, COLS)
        pv = prevf[:, n0:n0 + TF * S]
        pv = pv.reshape(C, TF, S).transpose(0, 2, 1).reshape(C * S, TF)
        pv2 = prevf[:, n0 + TF * S:n0 + PER_CORE].reshape(C * S2, 1)
        in_maps.append({
            "big": np.ascontiguousarray(big),
            "big2": np.ascontiguousarray(big2),
            "possb": np.ascontiguousarray(GAMMA * pv),
            "possb2": np.ascontiguousarray(GAMMA * pv2),
            "sel": sel,
            "sel2": sel2,
        })
    return in_maps


def _assemble_output(results):
    act = np.empty(UNITS, np.float32)
    for k in range(N_CORES):
        o = np.asarray(results[k]["out"])            # [S, T]
        loc = o[:, 0:TF].T.reshape(TF * S)           # unit n_local = 8t + s
        act[k * PER_CORE:k * PER_CORE + TF * S] = loc
        act[k * PER_CORE + TF * S:(k + 1) * PER_CORE] = o[0:S2, TF]
    out = np.broadcast_to(act.reshape(1, GX, GY), (C, GX, GY))
    return np.ascontiguousarray(out, dtype=np.float32)


def kernel(**inputs):
    nc = _get_program()
    in_maps = _prep_in_maps(inputs)
    res = run_bass_kernel_spmd(nc, in_maps, core_ids=list(range(N_CORES)))
    return _assemble_output(res.results)



# revision 4
# speedup vs baseline: 2.1318x; 2.1318x over previous
"""Trainium2 Bass kernel for nn_CortexNetwork (dense_cnn, memory-bound).

Reference computation:
    patches[c,i,j,u,v] = x[c, rx[i]+u, ry[j]+v]
    aff[i,j] = sum_{c,u,v} patches * Wa
    exc[i,j] = sum_c prev[c,i,j] * sum_{x,y} We[c,i,j,x,y]   (inh likewise, Wi)
    out      = broadcast_c(relu(aff + 0.9*exc - 0.9*inh))

Strategy: tensor-parallel over the 36x36=1296 grid units, 162 units per
core on 8 cores (padded to 168 = 21 groups of 8 so every tile is full
128-partition); every reduction is unit-local so there are no
collectives.  The kernel is HBM-bandwidth-bound, so all streamed data is
quantized to fp8_e4m3 on the host with sum-preserving rounding:

  * lateral We|-Wi rows (x64 scale) use error-diffusion rounding along
    each row, so the device's row sums match the f32 sums to ~1e-3;
  * afferent weights (x64) are rounded with the running product-sum
    carried against the fp8 patches (x16), GPTQ-style, so the device's
    dot products track the f32 products; patches are plain RTN fp8.

Measured end-to-end max-rel-error of this scheme on the reference inputs
is ~1.8e-3 (vs the 2e-2 gate) -- better than all-bf16, at half the
bytes.  Each (c, unit) row is 3744 B: lat 2592 | Wa 576 | patch 576,
one linear 479 KB DMA per 8-unit group.  Lateral row sums run on
ScalarE (activation, scale=0.9*prev/64, accum_out) for 10 groups and on
VectorE (tensor_reduce + per-partition mul) for 11; afferent rows are a
single fused VectorE tensor_tensor_reduce (multiply + accumulate).  The
16-channel sums run as 0/1-selector matmuls on the tensor engine (the
afferent selector carries the 1/1024 dequant scale), then relu.
"""

import numpy as np
import ml_dtypes

import concourse.bass as bass
import concourse.bacc as bacc
import concourse.mybir as mybir
from concourse import tile
from concourse.bass_utils import run_bass_kernel_spmd

N_CORES = 8
C = 16
GX = GY = 36
RF = 24
IMG = 64
GAMMA = 0.9

UNITS = GX * GY                  # 1296
PER_CORE = UNITS // N_CORES      # 162
S = 8                            # units per group (partition dim C*S=128)
T = 21                           # groups per core (168 units, 6 padded)
PADU = T * S                     # 168
FW = GX * GY                     # lateral free size per channel: 1296
FA = RF * RF                     # afferent free size per channel: 576
LCOL = 2 * FW                    # 2592 fp8 lateral bytes
COLS = LCOL + 2 * FA             # 3744 = lat | wa | patch
WSCALE = 64.0                    # fp8 scale for We/Wi/Wa
PSCALE = 16.0                    # fp8 scale for patches
# Groups whose lateral reduction runs on ScalarE; the rest (incl. the
# last, for a fast drain) go to VectorE.
ACT_GROUPS = frozenset((0, 2, 4, 6, 8, 10, 12, 14, 16, 18))

F8 = ml_dtypes.float8_e4m3

_PROGRAM_CACHE = {}


def _build_program():
    f32 = mybir.dt.float32
    f8 = mybir.dt.float8e4
    bf16 = mybir.dt.bfloat16
    AL = mybir.AluOpType
    AF = mybir.ActivationFunctionType
    AX = mybir.AxisListType

    nc = bacc.Bacc(
        "TRN2", target_bir_lowering=False, debug=False, num_devices=N_CORES
    )
    big_d = nc.dram_tensor("big", [T, 128, COLS], f8, kind="ExternalInput").ap()
    possb_d = nc.dram_tensor("possb", [128, T], f32, kind="ExternalInput").ap()
    sel_d = nc.dram_tensor("sel", [128, S], f32, kind="ExternalInput").ap()
    sela_d = nc.dram_tensor("sela", [128, S], f32, kind="ExternalInput").ap()
    out_d = nc.dram_tensor("out", [S, T], f32, kind="ExternalOutput").ap()

    with tile.TileContext(nc) as tc:
        with (
            tc.tile_pool(name="w", bufs=8) as wp,
            tc.tile_pool(name="cst", bufs=1) as cp,
            tc.tile_pool(name="junk", bufs=3) as jp,
            tc.tile_pool(name="acc", bufs=3) as accp,
            tc.tile_pool(name="fin", bufs=1) as fp,
            tc.tile_pool(name="ps", bufs=1, space="PSUM") as pp,
        ):
            possb = cp.tile([128, T], f32, tag="possb")
            sel = cp.tile([128, S], f32, tag="sel")
            sela = cp.tile([128, S], f32, tag="sela")
            plat = cp.tile([128, T], f32, tag="plat")
            paff = cp.tile([128, T], f32, tag="paff")
            nc.gpsimd.dma_start(possb[:], possb_d[:])
            nc.gpsimd.dma_start(sel[:], sel_d[:])
            nc.gpsimd.dma_start(sela[:], sela_d[:])

            for t in range(T):
                w = wp.tile([128, COLS], f8, tag="w")
                nc.sync.dma_start(w[:], big_d[t])
                pcol = possb[:, t:t + 1]
                if t in ACT_GROUPS:
                    j = jp.tile([128, LCOL], f32, tag="jlat")
                    nc.scalar.activation(
                        j[:], w[:, 0:LCOL], AF.Copy,
                        scale=pcol, accum_out=plat[:, t:t + 1],
                    )
                else:
                    r = accp.tile([128, 1], f32, tag="r")
                    nc.vector.tensor_reduce(
                        r[:], w[:, 0:LCOL], axis=AX.X, op=AL.add
                    )
                    nc.vector.tensor_mul(plat[:, t:t + 1], r[:], pcol)
                ja = jp.tile([128, FA], bf16, tag="jaff")
                nc.vector.scalar_tensor_tensor(
                    ja[:], w[:, LCOL:LCOL + FA], 1.0, w[:, LCOL + FA:COLS],
                    op0=AL.mult, op1=AL.mult,
                    accum_out=paff[:, t:t + 1],
                )

            psum = pp.tile([S, T], f32, tag="ps")
            nc.tensor.matmul(psum[:], sel[:], plat[:], start=True, stop=False)
            nc.tensor.matmul(psum[:], sela[:], paff[:], start=False, stop=True)

            res = fp.tile([S, T], f32, tag="res")
            nc.vector.tensor_scalar_max(res[:], psum[:], 0.0)
            nc.sync.dma_start(out_d[:], res[:])

    nc.compile()
    return nc


def _get_program():
    if "nc" not in _PROGRAM_CACHE:
        _PROGRAM_CACHE["nc"] = _build_program()
    return _PROGRAM_CACHE["nc"]


def _f8(v):
    return np.clip(v, -240.0, 240.0).astype(F8)


def _ed_rows(w, chunk):
    """fp8 quantize along the last axis with error-diffusion so each
    chunk's sum is preserved to ~one fp8 step."""
    r, n = w.shape
    wv = w.reshape(r * (n // chunk), chunk)
    q = np.empty(wv.shape, F8)
    carry = np.zeros(wv.shape[0], np.float32)
    for k in range(chunk):
        t = wv[:, k] + carry
        qk = _f8(t)
        q[:, k] = qk
        carry = t - qk.astype(np.float32)
    return q.reshape(r, n)


def _gptq_wa(wa_s, pq, t_s):
    """fp8-round scaled afferent weights with the running product-sum
    carried against the fp8 patches, so sum(q*pq) tracks sum(t_s)."""
    r, n = wa_s.shape
    pqf = pq.astype(np.float32)
    q = np.empty((r, n), F8)
    carry = np.zeros(r, np.float32)
    for k in range(n):
        tk = t_s[:, k] + carry
        pk = pqf[:, k]
        safe = np.where(pk == 0, 1.0, pk)
        v = np.where(pk != 0, tk / safe, wa_s[:, k])
        qk = _f8(v)
        q[:, k] = qk
        carry = tk - qk.astype(np.float32) * pk
    return q


def _prep_in_maps(inputs):
    x = np.asarray(inputs["x"], dtype=np.float32)
    prev = np.asarray(inputs["prev_activity"], dtype=np.float32).reshape(C, UNITS)
    wa = np.asarray(inputs["afferent_weights"], dtype=np.float32).reshape(C, UNITS, FA)
    we = np.asarray(inputs["ex_lateral_weights"], dtype=np.float32).reshape(C, UNITS, FW)
    wi = np.asarray(inputs["in_lateral_weights"], dtype=np.float32).reshape(C, UNITS, FW)
    rx = np.asarray(inputs["rx"]).astype(np.int64)
    ry = np.asarray(inputs["ry"]).astype(np.int64)

    u = np.arange(RF)
    ix = rx[:, None] + u                     # [GX, RF]
    iy = ry[:, None] + u                     # [GY, RF]
    px = x[:, ix, :]                         # [C, GX, RF, IMG]
    patches = px[:, :, :, iy]                # [C, GX, RF, GY, RF]
    patches = np.ascontiguousarray(patches.transpose(0, 1, 3, 2, 4))
    patches = patches.reshape(C * UNITS, FA)

    lat = np.concatenate([we, -wi], axis=2).reshape(C * UNITS, LCOL)
    lat_q = _ed_rows(lat * WSCALE, 324)                       # [C*U, 2592] f8
    pq = _f8(patches * PSCALE)                                # [C*U, 576] f8
    wa2 = wa.reshape(C * UNITS, FA)
    t_s = (wa2 * patches) * (WSCALE * PSCALE)
    wa_q = _gptq_wa(wa2 * WSCALE, pq, t_s)                    # [C*U, 576] f8

    blk = np.concatenate([lat_q, wa_q, pq], axis=1)           # [C*U, 3744] f8
    blk = blk.reshape(C, UNITS, COLS)
    prevf = prev * (GAMMA / WSCALE)

    sel = (np.arange(128)[:, None] % S == np.arange(S)[None, :]).astype(np.float32)
    sela = sel * np.float32(1.0 / (WSCALE * PSCALE))

    in_maps = []
    for k in range(N_CORES):
        n0 = k * PER_CORE
        b = np.zeros((C, PADU, COLS), F8)
        b[:, :PER_CORE] = blk[:, n0:n0 + PER_CORE]
        big = b.reshape(C, T, S, COLS).transpose(1, 0, 2, 3).reshape(T, 128, COLS)
        pv = np.zeros((C, PADU), np.float32)
        pv[:, :PER_CORE] = prevf[:, n0:n0 + PER_CORE]
        pv = pv.reshape(C, T, S).transpose(0, 2, 1).reshape(128, T)
        in_maps.append({
            "big": np.ascontiguousarray(big),
            "possb": np.ascontiguousarray(pv),
            "sel": sel,
            "sela": sela,
        })
    return in_maps


def _assemble_output(results):
    act = np.empty(UNITS, np.float32)
    for k in range(N_CORES):
        o = np.asarray(results[k]["out"])            # [S, T]
        act[k * PER_CORE:(k + 1) * PER_CORE] = o.T.reshape(PADU)[:PER_CORE]
    out = np.broadcast_to(act.reshape(1, GX, GY), (C, GX, GY))
    return np.ascontiguousarray(out, dtype=np.float32)


def kernel(**inputs):
    nc = _get_program()
    in_maps = _prep_in_maps(inputs)
    res = run_bass_kernel_spmd(nc, in_maps, core_ids=list(range(N_CORES)))
    return _assemble_output(res.results)


# revision 5
# speedup vs baseline: 2.9923x; 1.4036x over previous
"""Trainium2 Bass kernel for nn_CortexNetwork (dense_cnn, memory-bound).

Reference computation:
    patches[c,i,j,u,v] = x[c, rx[i]+u, ry[j]+v]
    aff[i,j] = sum_{c,u,v} patches * Wa
    exc[i,j] = sum_c prev[c,i,j] * sum_{x,y} We[c,i,j,x,y]   (inh likewise, Wi)
    out      = broadcast_c(relu(aff + 0.9*exc - 0.9*inh))

Strategy: tensor-parallel over the 36x36=1296 grid units, 162 units per
core on 8 cores (padded to 168 = 21 groups of 8 so every tile is a full
128 partitions = 16 channels x 8 units); every reduction is unit-local
so there are no collectives.  The kernel is HBM-bandwidth-bound, so all
streamed data is fp8_e4m3, quantized on the host with sum-preserving
rounding:

  * lateral We|-Wi rows (x64 scale) use error-diffusion rounding along
    each row, so the device's row sums match the f32 sums to ~1e-3;
  * afferent weights (x64) are rounded with the running product-sum
    carried against the fp8 patches (x16), GPTQ-style, so the device's
    dot products track the f32 products; patches are plain RTN fp8.

Measured end-to-end max-rel-error on the reference inputs is ~1.8e-3
(vs the 2e-2 gate) at 1/4 the f32 bytes.

The 2592-wide lateral row sums are split across three engines so no one
engine exceeds the ~28us DMA stream time: 16 groups go to the tensor
engine as 21 chained LDWEIGHTS(128x128 fp8 lateral block) + 1-column
matmuls against a constant fp8 ones vector, accumulating each group's
per-(c,unit) row sum into one PSUM column (lateral data for these
groups is host-transposed, padded 2592->2688); 5 groups go to ScalarE
as activation(Copy, scale=0.9*prev/64, accum_out).  VectorE only runs
the fused afferent multiply-reduce (scalar_tensor_tensor) per group,
the 0.9*prev/64 multiply for the PE block, and the final relu.  The
16-channel sums are 0/1-selector matmuls on PE (the afferent selector
carries the 1/1024 dequant scale).
"""

import numpy as np
import ml_dtypes

import concourse.bass as bass
import concourse.bacc as bacc
import concourse.mybir as mybir
from concourse import tile
from concourse.bass_utils import run_bass_kernel_spmd

N_CORES = 8
C = 16
GX = GY = 36
RF = 24
IMG = 64
GAMMA = 0.9

UNITS = GX * GY                  # 1296
PER_CORE = UNITS // N_CORES      # 162
S = 8                            # units per group (partition dim C*S=128)
T = 21                           # groups per core (168 units, 6 padded)
PADU = T * S                     # 168
FW = GX * GY                     # lateral free size per channel: 1296
FA = RF * RF                     # afferent free size per channel: 576
LCOL = 2 * FW                    # 2592 lateral elems per (c,unit)
KC = 21                          # 128-chunks of the padded lateral dim
LPAD = KC * 128                  # 2688
COLS_A = LCOL + 2 * FA           # 3744  (ScalarE-group record)
COLS_P = LPAD + 2 * FA           # 3840  (PE-group record, lateral transposed)
WSCALE = 64.0                    # fp8 scale for We/Wi/Wa
PSCALE = 16.0                    # fp8 scale for patches
ACT_SET = (2, 6, 10, 14, 18)     # groups whose lateral runs on ScalarE
PE_GROUPS = tuple(t for t in range(T) if t not in ACT_SET)
NPE = len(PE_GROUPS)             # 16
# output column of each group: PE block first, then ACT block
COL_OF = {t: i for i, t in enumerate(PE_GROUPS)}
COL_OF.update({t: NPE + j for j, t in enumerate(ACT_SET)})

F8 = ml_dtypes.float8_e4m3

_PROGRAM_CACHE = {}


def _build_program():
    f32 = mybir.dt.float32
    f8 = mybir.dt.float8e4
    bf16 = mybir.dt.bfloat16
    AL = mybir.AluOpType
    AF = mybir.ActivationFunctionType

    nc = bacc.Bacc(
        "TRN2", target_bir_lowering=False, debug=False, num_devices=N_CORES
    )
    bigp_d = nc.dram_tensor("bigp", [NPE, 128, COLS_P], f8, kind="ExternalInput").ap()
    biga_d = nc.dram_tensor("biga", [T - NPE, 128, COLS_A], f8, kind="ExternalInput").ap()
    possb_d = nc.dram_tensor("possb", [128, T], f32, kind="ExternalInput").ap()
    sel_d = nc.dram_tensor("sel", [128, S], f32, kind="ExternalInput").ap()
    sela_d = nc.dram_tensor("sela", [128, S], f32, kind="ExternalInput").ap()
    out_d = nc.dram_tensor("out", [S, T], f32, kind="ExternalOutput").ap()

    with tile.TileContext(nc) as tc:
        with (
            tc.tile_pool(name="wp", bufs=6) as wpp,
            tc.tile_pool(name="wa", bufs=3) as wap,
            tc.tile_pool(name="cst", bufs=1) as cp,
            tc.tile_pool(name="junk", bufs=3) as jp,
            tc.tile_pool(name="fin", bufs=1) as fp,
            tc.tile_pool(name="ps", bufs=1, space="PSUM") as pp,
        ):
            possb = cp.tile([128, T], f32, tag="possb")
            sel = cp.tile([128, S], f32, tag="sel")
            sela = cp.tile([128, S], f32, tag="sela")
            ones = cp.tile([128, 1], f8, tag="ones")
            plat = cp.tile([128, T], f32, tag="plat")
            paff = cp.tile([128, T], f32, tag="paff")
            nc.gpsimd.dma_start(possb[:], possb_d[:])
            nc.gpsimd.dma_start(sel[:], sel_d[:])
            nc.gpsimd.dma_start(sela[:], sela_d[:])
            nc.vector.memset(ones[:], 1.0)

            pslat = pp.tile([128, NPE], f32, tag="pslat")

            for t in range(T):
                col = COL_OF[t]
                if t in ACT_SET:
                    w = wap.tile([128, COLS_A], f8, tag="wa")
                    nc.sync.dma_start(w[:], biga_d[col - NPE])
                    j = jp.tile([128, LCOL], f32, tag="jlat")
                    nc.scalar.activation(
                        j[:], w[:, 0:LCOL], AF.Copy,
                        scale=possb[:, col:col + 1],
                        accum_out=plat[:, col:col + 1],
                    )
                    aoff = LCOL
                else:
                    w = wpp.tile([128, COLS_P], f8, tag="wp")
                    nc.sync.dma_start(w[:], bigp_d[col])
                    for k in range(KC):
                        nc.tensor.matmul(
                            pslat[:, col:col + 1],
                            w[:, 128 * k:128 * (k + 1)], ones[:],
                            start=(k == 0), stop=(k == KC - 1),
                        )
                    aoff = LPAD
                ja = jp.tile([128, FA], bf16, tag="jaff")
                nc.vector.scalar_tensor_tensor(
                    ja[:], w[:, aoff:aoff + FA], 1.0, w[:, aoff + FA:aoff + 2 * FA],
                    op0=AL.mult, op1=AL.mult,
                    accum_out=paff[:, col:col + 1],
                )

            nc.vector.tensor_mul(
                plat[:, 0:NPE], pslat[:], possb[:, 0:NPE]
            )

            psum = pp.tile([S, T], f32, tag="ps")
            nc.tensor.matmul(psum[:], sel[:], plat[:], start=True, stop=False)
            nc.tensor.matmul(psum[:], sela[:], paff[:], start=False, stop=True)

            res = fp.tile([S, T], f32, tag="res")
            nc.vector.tensor_scalar_max(res[:], psum[:], 0.0)
            nc.sync.dma_start(out_d[:], res[:])

    nc.compile()
    return nc


def _get_program():
    if "nc" not in _PROGRAM_CACHE:
        _PROGRAM_CACHE["nc"] = _build_program()
    return _PROGRAM_CACHE["nc"]


def _f8(v):
    return np.clip(v, -240.0, 240.0).astype(F8)


def _ed_rows(w, chunk):
    """fp8 quantize along the last axis with error-diffusion so each
    chunk's sum is preserved to ~one fp8 step."""
    r, n = w.shape
    wv = w.reshape(r * (n // chunk), chunk)
    q = np.empty(wv.shape, F8)
    carry = np.zeros(wv.shape[0], np.float32)
    for k in range(chunk):
        t = wv[:, k] + carry
        qk = _f8(t)
        q[:, k] = qk
        carry = t - qk.astype(np.float32)
    return q.reshape(r, n)


def _gptq_wa(wa_s, pq, t_s):
    """fp8-round scaled afferent weights with the running product-sum
    carried against the fp8 patches, so sum(q*pq) tracks sum(t_s)."""
    r, n = wa_s.shape
    pqf = pq.astype(np.float32)
    q = np.empty((r, n), F8)
    carry = np.zeros(r, np.float32)
    for k in range(n):
        tk = t_s[:, k] + carry
        pk = pqf[:, k]
        safe = np.where(pk == 0, 1.0, pk)
        v = np.where(pk != 0, tk / safe, wa_s[:, k])
        qk = _f8(v)
        q[:, k] = qk
        carry = tk - qk.astype(np.float32) * pk
    return q


def _prep_in_maps(inputs):
    x = np.asarray(inputs["x"], dtype=np.float32)
    prev = np.asarray(inputs["prev_activity"], dtype=np.float32).reshape(C, UNITS)
    wa = np.asarray(inputs["afferent_weights"], dtype=np.float32).reshape(C, UNITS, FA)
    we = np.asarray(inputs["ex_lateral_weights"], dtype=np.float32).reshape(C, UNITS, FW)
    wi = np.asarray(inputs["in_lateral_weights"], dtype=np.float32).reshape(C, UNITS, FW)
    rx = np.asarray(inputs["rx"]).astype(np.int64)
    ry = np.asarray(inputs["ry"]).astype(np.int64)

    u = np.arange(RF)
    ix = rx[:, None] + u                     # [GX, RF]
    iy = ry[:, None] + u                     # [GY, RF]
    px = x[:, ix, :]                         # [C, GX, RF, IMG]
    patches = px[:, :, :, iy]                # [C, GX, RF, GY, RF]
    patches = np.ascontiguousarray(patches.transpose(0, 1, 3, 2, 4))
    patches = patches.reshape(C * UNITS, FA)

    lat = np.concatenate([we, -wi], axis=2).reshape(C * UNITS, LCOL)
    lat_q = _ed_rows(lat * WSCALE, 324)                       # [C*U, 2592] f8
    pq = _f8(patches * PSCALE)                                # [C*U, 576] f8
    wa2 = wa.reshape(C * UNITS, FA)
    t_s = (wa2 * patches) * (WSCALE * PSCALE)
    wa_q = _gptq_wa(wa2 * WSCALE, pq, t_s)                    # [C*U, 576] f8

    lat_q = lat_q.reshape(C, UNITS, LCOL)
    affcat = np.concatenate(
        [wa_q.reshape(C, UNITS, FA), pq.reshape(C, UNITS, FA)], axis=2
    )                                                          # [C, U, 1152]
    prevf = prev * (GAMMA / WSCALE)

    sel = (np.arange(128)[:, None] % S == np.arange(S)[None, :]).astype(np.float32)
    sela = sel * np.float32(1.0 / (WSCALE * PSCALE))

    in_maps = []
    for kcore in range(N_CORES):
        n0 = kcore * PER_CORE
        # per-group padded [C, PADU, .] slices for this core
        lq = np.zeros((C, PADU, LCOL), F8)
        lq[:, :PER_CORE] = lat_q[:, n0:n0 + PER_CORE]
        af = np.zeros((C, PADU, 2 * FA), F8)
        af[:, :PER_CORE] = affcat[:, n0:n0 + PER_CORE]
        # partition-major [T, 128, .] with row p = c*S + s
        lqg = lq.reshape(C, T, S, LCOL).transpose(1, 0, 2, 3).reshape(T, 128, LCOL)
        afg = af.reshape(C, T, S, 2 * FA).transpose(1, 0, 2, 3).reshape(T, 128, 2 * FA)

        bigp = np.zeros((NPE, 128, COLS_P), F8)
        biga = np.zeros((T - NPE, 128, COLS_A), F8)
        for t in range(T):
            col = COL_OF[t]
            if t in ACT_SET:
                biga[col - NPE, :, 0:LCOL] = lqg[t]
                biga[col - NPE, :, LCOL:COLS_A] = afg[t]
            else:
                # transpose lateral: tile[p, 128k+f] = lat[f, 128k+p], zero pad
                lt = np.zeros((128, LPAD), F8)
                lt[:, 0:LCOL] = lqg[t]
                bigp[col, :, 0:LPAD] = (
                    lt.reshape(128, KC, 128).transpose(2, 1, 0).reshape(128, LPAD)
                )
                bigp[col, :, LPAD:COLS_P] = afg[t]

        pv = np.zeros((C, PADU), np.float32)
        pv[:, :PER_CORE] = prevf[:, n0:n0 + PER_CORE]
        pv = pv.reshape(C, T, S).transpose(0, 2, 1).reshape(128, T)
        pvp = np.empty_like(pv)
        for t in range(T):
            pvp[:, COL_OF[t]] = pv[:, t]
        in_maps.append({
            "bigp": np.ascontiguousarray(bigp),
            "biga": np.ascontiguousarray(biga),
            "possb": np.ascontiguousarray(pvp),
            "sel": sel,
            "sela": sela,
        })
    return in_maps


def _assemble_output(results):
    act = np.empty(UNITS, np.float32)
    for kcore in range(N_CORES):
        o = np.asarray(results[kcore]["out"])            # [S, T] permuted cols
        for t in range(T):
            n0 = kcore * PER_CORE + t * S
            if n0 >= (kcore + 1) * PER_CORE:
                break
            nn = min(S, (kcore + 1) * PER_CORE - n0)
            act[n0:n0 + nn] = o[:nn, COL_OF[t]]
    out = np.broadcast_to(act.reshape(1, GX, GY), (C, GX, GY))
    return np.ascontiguousarray(out, dtype=np.float32)


def kernel(**inputs):
    nc = _get_program()
    in_maps = _prep_in_maps(inputs)
    res = run_bass_kernel_spmd(nc, in_maps, core_ids=list(range(N_CORES)))
    return _assemble_output(res.results)
